# revision 10
# baseline (speedup 1.0000x reference)
"""Trainium2 Bass kernel for nn_DirectionVarEntropy.

Computes, per 14x14 patch and channel:
  - pixel-value entropy (256-bin histogram of round(x*255))
  - direction variance psi of 3x3-DCT sliding-window directional stds
  - richness = mean_c(psi_m * entropy)  ->  output (B, Hp, Wp)

Sharding: pure data parallel over batch, 2 images per core on 8 cores.

Per-core layout: 2048 spatial patches x 3 channels = 6144 patch-channels,
mapped to [128 partitions x 48 free segments]; seg s = t*3 + c where
t = spatial_patch // 128, partition p = spatial_patch % 128.

Histogram: per (seg, bin) fused DVE tensor_scalar(is_equal, accum_out) on
bf16 pixel codes (4x DVE mode) -- counts land in SBUF, entropy tail uses
ACT Ln + fused tensor_tensor_reduce.

DCT part: explicit 9 coefficient planes via separable 3-tap convs
(tensor_scalar + scalar_tensor_tensor with shifted access patterns),
group sums / stds / psi in fp32 on DVE with ACT doing squares & sqrts.
"""

import functools

import numpy as np

import concourse.bacc as bacc
import concourse.bass as bass
import concourse.mybir as mybir
from concourse import bass_utils
from concourse.tile import TileContext

P = 128
PH = 14
NWIN = 12          # sliding 3x3 positions per axis
NPIX = PH * PH     # 196
BINS = 256
LN2 = 0.6931471805599453
F32 = mybir.dt.float32
BF16 = mybir.dt.bfloat16
ALU = mybir.AluOpType
ACTF = mybir.ActivationFunctionType

# problem shape (hardcoded per contract)
B_FULL, C, H, W = 16, 3, 448, 448
N_CORES = 8
B_CORE = B_FULL // N_CORES      # 2
HP = H // PH                    # 32
T_BLKS = B_CORE * HP * HP // P  # 16 t-blocks of 128 spatial patches
SEGS = T_BLKS * C               # 48


def _build(dct_flat: tuple, segs: int = SEGS, bins: int = BINS,
           nb: int = 2) -> bass.Bass:
    """Build the SPMD single-core program. dct_flat: 9 floats, row major."""
    D = np.asarray(dct_flat, np.float64).reshape(3, 3)
    nc = bacc.Bacc("TRN2", debug=False, enable_asserts=False)

    x_d = nc.dram_tensor("x", (B_CORE, C, H, W), F32, kind="ExternalInput")
    out_d = nc.dram_tensor("out", (B_CORE, HP, HP), F32, kind="ExternalOutput")
    # (b, c, hp, i, wp, j) view of DRAM input, reordered to (b c hp wp i j)
    xv = x_d.ap().rearrange("b c (hp i) (wp j) -> b c hp wp i j", i=PH, j=PH)
    ov = out_d.ap()

    n_blocks = (segs + nb - 1) // nb

    with TileContext(nc) as tc:
        with tc.tile_pool(name="persist", bufs=1) as pp, \
             tc.tile_pool(name="work", bufs=2) as wp, \
             tc.tile_pool(name="ent", bufs=2) as ep:
            X = pp.tile([P, segs, PH, PH], F32)
            Xf = X.rearrange("p s i j -> p (s i j)")
            TMP = pp.tile([P, (segs // 8) * NPIX], F32)
            PI = pp.tile([P, segs, NPIX], BF16)
            PIf = PI.rearrange("p s k -> p (s k)")
            dummy = pp.tile([P, NPIX], BF16)
            wdum = pp.tile([P, bins], F32)
            pdum = pp.tile([P, NWIN * NWIN], F32)
            psi_acc = pp.tile([P, segs], F32)
            e_acc = pp.tile([P, segs], F32)
            rich = pp.tile([P, segs], F32)
            rich3 = rich.rearrange("p (t c) -> p t c", c=C)
            tsum = pp.tile([P, segs // C], F32)
            osb = pp.tile([P, segs // C], F32)

            # ---- input DMAs: per (t, c, p1) a [32, 14, 14] strided load ----
            for t in range(T_BLKS):
                b = t // (T_BLKS // B_CORE)
                hp0 = (t % (T_BLKS // B_CORE)) * 4
                for c in range(C):
                    s = t * C + c
                    for p1 in range(4):
                        nc.sync.dma_start(
                            X[p1 * 32:(p1 + 1) * 32, s],
                            xv[b, c, hp0 + p1],
                        )
            # Per-DMA same-engine absorber copies: each waits on exactly one
            # DMA queue semaphore; all downstream DVE reads of X then order
            # behind these in program order (no multi-sem waits, which
            # overflow the ISA sync-wait slots).
            for t in range(T_BLKS):
                for c in range(C):
                    s = t * C + c
                    for p1 in range(4):
                        sl = X[p1 * 32:(p1 + 1) * 32, s]
                        nc.vector.tensor_copy(sl, sl)

            # ---- quantize: pi = round(x*255) via the 2^23 RNE trick ----
            TWO23 = float(2 ** 23)
            qch = (segs // 8) * NPIX
            for q in range(8):
                nc.vector.tensor_scalar(
                    TMP, Xf[:, q * qch:(q + 1) * qch], 255.0, TWO23,
                    ALU.mult, ALU.add)
                nc.vector.tensor_scalar(
                    PIf[:, q * qch:(q + 1) * qch], TMP, TWO23, None,
                    ALU.subtract)

            d = [[float(D[r, c]) for c in range(3)] for r in range(3)]

            for blk in range(n_blocks):
                s0 = blk * nb
                sn = min(nb, segs - s0)
                # conv tiles for this block
                V = [wp.tile([P, nb, NWIN, PH], F32, tag=f"V{r}", name=f"V{r}")
                     for r in range(3)]
                Y = [[wp.tile([P, nb, NWIN, NWIN], F32, tag=f"Y{r}{c}", name=f"Y{r}{c}")
                      for c in range(3)] for r in range(3)]
                xb = X[:, s0:s0 + sn]

                # vertical convs V_r(i,j) = sum_k D[r,k] x(i+k, j)
                for r in range(3):
                    vb = V[r][:, :sn]
                    nc.vector.tensor_scalar(
                        vb, xb[:, :, 0:NWIN, :], d[r][0], None, ALU.mult)
                    for k in (1, 2):
                        nc.vector.scalar_tensor_tensor(
                            vb, xb[:, :, k:k + NWIN, :], d[r][k], vb,
                            ALU.mult, ALU.add)
                # horizontal convs Y_rc(i,j) = sum_l D[c,l] V_r(i, j+l)
                for r in range(3):
                    vb = V[r][:, :sn]
                    for c in range(3):
                        yb = Y[r][c][:, :sn]
                        nc.vector.tensor_scalar(
                            yb, vb[:, :, :, 0:NWIN], d[c][0], None, ALU.mult)
                        for l in (1, 2):
                            nc.vector.scalar_tensor_tensor(
                                yb, vb[:, :, :, l:l + NWIN], d[c][l], yb,
                                ALU.mult, ALU.add)

                # group sums of Y (pre-square): rows, cols, diag, anti-diag
                GROUPS = (
                    [[(r, 0), (r, 1), (r, 2)] for r in range(3)]       # rows
                    + [[(0, c), (1, c), (2, c)] for c in range(3)]     # cols
                    + [[(0, 0), (1, 1), (2, 2)],                       # diag
                       [(0, 2), (1, 1), (2, 0)]]                       # anti
                )
                M = [wp.tile([P, nb, NWIN, NWIN], F32, tag=f"M{g}", name=f"M{g}")
                     for g in range(8)]
                SS = [wp.tile([P, nb, NWIN, NWIN], F32, tag=f"SS{g}", name=f"SS{g}")
                      for g in range(8)]
                for g, mem in enumerate(GROUPS):
                    mb = M[g][:, :sn]
                    (r0, c0), (r1, c1), (r2, c2) = mem
                    nc.vector.tensor_add(
                        mb, Y[r0][c0][:, :sn], Y[r1][c1][:, :sn])
                    nc.vector.tensor_add(mb, mb, Y[r2][c2][:, :sn])
                    # Msq = (M/3)^2 in place
                    nc.scalar.activation(mb, mb, ACTF.Square, scale=1.0 / 3)
                # squares of Y in place
                for r in range(3):
                    for c in range(3):
                        yb = Y[r][c][:, :sn]
                        nc.scalar.activation(yb, yb, ACTF.Square)
                for g, mem in enumerate(GROUPS):
                    sb = SS[g][:, :sn]
                    (r0, c0), (r1, c1), (r2, c2) = mem
                    nc.vector.tensor_add(
                        sb, Y[r0][c0][:, :sn], Y[r1][c1][:, :sn])
                    nc.vector.tensor_add(sb, sb, Y[r2][c2][:, :sn])
                    # std^2 = SS/3 - (M/3)^2, clamp, sqrt -> sigma in SS tile
                    nc.vector.scalar_tensor_tensor(
                        sb, sb, 1.0 / 3, M[g][:, :sn], ALU.mult, ALU.subtract)
                    nc.vector.tensor_scalar_max(sb, sb, 0.0)
                    nc.scalar.activation(sb, sb, ACTF.Sqrt)

                U1 = wp.tile([P, nb, NWIN, NWIN], F32, tag="U1", name="U1")
                U2 = wp.tile([P, nb, NWIN, NWIN], F32, tag="U2", name="U2")
                t1 = wp.tile([P, nb, NWIN, NWIN], F32, tag="t1", name="t1")
                t2 = wp.tile([P, nb, NWIN, NWIN], F32, tag="t2", name="t2")
                A = wp.tile([P, nb, NWIN, NWIN], F32, tag="A", name="A")
                sum2 = wp.tile([P, nb, NWIN, NWIN], F32, tag="sum2", name="sum2")
                aq = wp.tile([P, nb, NWIN, NWIN], F32, tag="aq", name="aq")
                s_t = wp.tile([P, nb, NWIN, NWIN], F32, tag="s_t", name="s_t")
                ssq = wp.tile([P, nb, NWIN, NWIN], F32, tag="ssq", name="ssq")
                rinv = wp.tile([P, nb, NWIN, NWIN], F32, tag="rinv", name="rinv")
                psi = wp.tile([P, nb, NWIN, NWIN], F32, tag="psi", name="psi")
                u1, u2 = U1[:, :sn], U2[:, :sn]
                tb1, tb2 = t1[:, :sn], t2[:, :sn]
                ab = A[:, :sn]
                s2b, aqb = sum2[:, :sn], aq[:, :sn]
                stb, ssqb, rb, psib = (s_t[:, :sn], ssq[:, :sn],
                                       rinv[:, :sn], psi[:, :sn])
                sig = [SS[g][:, :sn] for g in range(8)]

                nc.vector.tensor_add(u1, sig[0], sig[1])
                nc.vector.tensor_add(u1, u1, sig[2])
                nc.vector.tensor_add(u2, sig[3], sig[4])
                nc.vector.tensor_add(u2, u2, sig[5])
                # A = U1/3 + U2/3 + sig6 + sig7
                nc.vector.scalar_tensor_tensor(
                    tb1, u1, 1.0 / 3, sig[6], ALU.mult, ALU.add)
                nc.vector.scalar_tensor_tensor(
                    tb2, u2, 1.0 / 3, sig[7], ALU.mult, ALU.add)
                nc.vector.tensor_add(ab, tb1, tb2)
                # sum of squared directional stds
                nc.scalar.activation(u1, u1, ACTF.Square, scale=1.0 / 3)
                nc.scalar.activation(u2, u2, ACTF.Square, scale=1.0 / 3)
                nc.scalar.activation(sig[6], sig[6], ACTF.Square)
                nc.scalar.activation(sig[7], sig[7], ACTF.Square)
                nc.vector.tensor_add(tb1, u1, u2)
                nc.vector.tensor_add(tb2, sig[6], sig[7])
                nc.vector.tensor_add(s2b, tb1, tb2)
                # psi = (sum2 - A^2/4) / (3 * (A/4 + 1e-8)^2)
                nc.scalar.activation(aqb, ab, ACTF.Square, scale=0.5)
                nc.vector.tensor_sub(s2b, s2b, aqb)
                nc.vector.tensor_scalar(
                    stb, ab, 0.25, 1e-8, ALU.mult, ALU.add)
                nc.scalar.activation(ssqb, stb, ACTF.Square)
                nc.vector.reciprocal(rb, ssqb)
                nc.vector.scalar_tensor_tensor(
                    psib, s2b, 1.0 / 3, rb, ALU.mult, ALU.mult)
                # psi_m accumulate per seg
                for i in range(sn):
                    s = s0 + i
                    nc.vector.tensor_scalar(
                        pdum, psib[:, i].rearrange("p i j -> p (i j)"),
                        1.0, None, ALU.mult, ALU.add,
                        accum_out=psi_acc[:, s:s + 1])

                # ---- entropy for the segs of this block ----
                for i in range(sn):
                    s = s0 + i
                    cnt = ep.tile([P, bins], F32, tag="cnt", name="cnt")
                    u = ep.tile([P, bins], F32, tag="u", name="u")
                    ul = ep.tile([P, bins], F32, tag="ul", name="ul")
                    for b in range(bins):
                        nc.vector.tensor_scalar(
                            dummy, PI[:, s], float(b), None,
                            ALU.is_equal, ALU.add,
                            accum_out=cnt[:, b:b + 1])
                    nc.vector.tensor_scalar(
                        u, cnt, 1.0 / NPIX, 1e-10, ALU.mult, ALU.add)
                    nc.vector.tensor_scalar(
                        ul, cnt, 1.0 / NPIX, 1e-3, ALU.mult, ALU.max)
                    nc.scalar.activation(ul, ul, ACTF.Ln)
                    nc.vector.tensor_mul(u, u, ul)
                    nc.vector.tensor_scalar(
                        wdum, u, 1.0, None, ALU.mult, ALU.add,
                        accum_out=e_acc[:, s:s + 1])

            # ---- richness = psi_m * entropy, mean over channels ----
            cmul = -1.0 / (NWIN * NWIN * LN2)
            nc.vector.scalar_tensor_tensor(
                rich, psi_acc, cmul, e_acc, ALU.mult, ALU.mult)
            nc.vector.tensor_add(tsum, rich3[:, :, 0], rich3[:, :, 1])
            nc.vector.tensor_add(tsum, tsum, rich3[:, :, 2])
            nc.vector.tensor_scalar(osb, tsum, 1.0 / C, None, ALU.mult)

            # ---- output DMAs ----
            for t in range(T_BLKS):
                b = t // (T_BLKS // B_CORE)
                hp0 = (t % (T_BLKS // B_CORE)) * 4
                nc.sync.dma_start(ov[b, hp0:hp0 + 4], osb[:, t:t + 1])

    nc.compile()
    return nc


@functools.lru_cache(maxsize=4)
def _build_cached(dct_flat: tuple) -> bass.Bass:
    return _build(dct_flat)


def kernel(x, dct_matrix):
    x = np.ascontiguousarray(np.asarray(x, dtype=np.float32))
    D = np.asarray(dct_matrix, dtype=np.float32)
    assert x.shape == (B_FULL, C, H, W), x.shape
    nc = _build_cached(tuple(float(v) for v in D.flatten()))
    in_maps = [
        {"x": np.ascontiguousarray(x[i * B_CORE:(i + 1) * B_CORE])}
        for i in range(N_CORES)
    ]
    res = bass_utils.run_bass_kernel_spmd(
        nc, in_maps, core_ids=list(range(N_CORES)))
    out = np.concatenate([r["out"] for r in res.results], axis=0)
    return out.astype(np.float32)


# revision 28
# speedup vs baseline: 1.5404x; 1.5404x over previous
"""Trainium2 Bass kernel for nn_DirectionVarEntropy.

Computes, per 14x14 patch and channel:
  - pixel-value entropy (256-bin histogram of round(x*255))
  - direction variance psi of 3x3-DCT sliding-window directional stds
  - richness = mean_c(psi_m * entropy)  ->  output (B, Hp, Wp)

Sharding: pure data parallel over batch, 2 images per core on 8 cores.

Per-core layout: 2048 spatial patches x 3 channels = 6144 patch-channels,
mapped to [128 partitions x 48 free segments]; seg s = t*3 + c where
t = spatial_patch // 128, partition p = spatial_patch % 128.

Histogram: per (seg, bin) fused DVE tensor_scalar(is_equal, accum_out) on
bf16 pixel codes (4x DVE mode) -- counts land in SBUF, entropy tail uses
ACT Ln + fused tensor_tensor_reduce.

DCT part: explicit 9 coefficient planes via separable 3-tap convs
(tensor_scalar + scalar_tensor_tensor with shifted access patterns),
group sums / stds / psi in fp32 on DVE with ACT doing squares & sqrts.
"""

import functools

import numpy as np

import concourse.bacc as bacc
import concourse.bass as bass
import concourse.mybir as mybir
from concourse import bass_utils
from concourse.tile import TileContext

P = 128
PH = 14
NWIN = 12          # sliding 3x3 positions per axis
NPIX = PH * PH     # 196
BINS = 256
LN2 = 0.6931471805599453
F32 = mybir.dt.float32
BF16 = mybir.dt.bfloat16
ALU = mybir.AluOpType
ACTF = mybir.ActivationFunctionType

# problem shape (hardcoded per contract)
B_FULL, C, H, W = 16, 3, 448, 448
N_CORES = 8
B_CORE = B_FULL // N_CORES      # 2
HP = H // PH                    # 32
T_BLKS = B_CORE * HP * HP // P  # 16 t-blocks of 128 spatial patches
SEGS = T_BLKS * C               # 48


def _build(dct_flat: tuple, segs: int = SEGS, bins: int = BINS,
           nb: int = 2, act_bins: int = 32) -> bass.Bass:
    """Build the SPMD single-core program. dct_flat: 9 floats, row major."""
    D = np.asarray(dct_flat, np.float64).reshape(3, 3)
    nc = bacc.Bacc("TRN2", debug=False, enable_asserts=False)

    x_d = nc.dram_tensor("x", (B_CORE, C, H, W), F32, kind="ExternalInput")
    out_d = nc.dram_tensor("out", (B_CORE, HP, HP), F32, kind="ExternalOutput")
    # (b, c, hp, i, wp, j) view of DRAM input, reordered to (b c hp wp i j)
    xv = x_d.ap().rearrange("b c (hp i) (wp j) -> b c hp wp i j", i=PH, j=PH)
    ov = out_d.ap()

    n_blocks = (segs + nb - 1) // nb

    with TileContext(nc) as tc:
        with tc.tile_pool(name="persist", bufs=1) as pp:
            X = pp.tile([P, segs, PH, PH], F32)
            Xf = X.rearrange("p s i j -> p (s i j)")
            TMP = pp.tile([P, (segs // 8) * NPIX], F32)
            dummy = pp.tile([P, NPIX], BF16)
            wdum = pp.tile([P, bins], F32)
            pdum = pp.tile([P, NWIN * NWIN], F32)
            psi_acc = pp.tile([P, segs], F32)
            e_acc = pp.tile([P, segs], F32)
            rich = pp.tile([P, segs], F32)
            rich3 = rich.rearrange("p (t c) -> p t c", c=C)
            tsum = pp.tile([P, segs // C], F32)
            osb = pp.tile([P, segs // C], F32)

            # ---- input DMAs: per (t, c, p1) a [32, 14, 14] strided load ----
            for t in range(T_BLKS):
                b = t // (T_BLKS // B_CORE)
                hp0 = (t % (T_BLKS // B_CORE)) * 4
                for c in range(C):
                    s = t * C + c
                    for p1 in range(4):
                        nc.sync.dma_start(
                            X[p1 * 32:(p1 + 1) * 32, s],
                            xv[b, c, hp0 + p1],
                        )
            # Per-DMA same-engine absorber copies: each waits on exactly one
            # DMA queue semaphore; all downstream DVE reads of X then order
            # behind these in program order (no multi-sem waits, which
            # overflow the ISA sync-wait slots).
            for t in range(T_BLKS):
                for c in range(C):
                    s = t * C + c
                    for p1 in range(4):
                        sl = X[p1 * 32:(p1 + 1) * 32, s]
                        nc.vector.tensor_copy(sl, sl)

            d = [[float(D[r, c]) for c in range(3)] for r in range(3)]

            wp_ctx = tc.tile_pool(name="work", bufs=2)
            wp = wp_ctx.__enter__()
            for blk in range(n_blocks):
                s0 = blk * nb
                sn = min(nb, segs - s0)
                # conv tiles for this block
                V = [wp.tile([P, nb, NWIN, PH], F32, tag=f"V{r}", name=f"V{r}")
                     for r in range(3)]
                Y = [[wp.tile([P, nb, NWIN, NWIN], F32, tag=f"Y{r}{c}", name=f"Y{r}{c}")
                      for c in range(3)] for r in range(3)]
                xb = X[:, s0:s0 + sn]

                # vertical convs V_r(i,j) = sum_k D[r,k] x(i+k, j)
                for r in range(3):
                    vb = V[r][:, :sn]
                    nc.vector.tensor_scalar(
                        vb, xb[:, :, 0:NWIN, :], d[r][0], None, ALU.mult)
                    for k in (1, 2):
                        nc.vector.scalar_tensor_tensor(
                            vb, xb[:, :, k:k + NWIN, :], d[r][k], vb,
                            ALU.mult, ALU.add)
                # horizontal convs Y_rc(i,j) = sum_l D[c,l] V_r(i, j+l)
                for r in range(3):
                    vb = V[r][:, :sn]
                    for c in range(3):
                        yb = Y[r][c][:, :sn]
                        nc.vector.tensor_scalar(
                            yb, vb[:, :, :, 0:NWIN], d[c][0], None, ALU.mult)
                        for l in (1, 2):
                            nc.vector.scalar_tensor_tensor(
                                yb, vb[:, :, :, l:l + NWIN], d[c][l], yb,
                                ALU.mult, ALU.add)

                # group sums of Y (pre-square): rows, cols, diag, anti-diag
                GROUPS = (
                    [[(r, 0), (r, 1), (r, 2)] for r in range(3)]       # rows
                    + [[(0, c), (1, c), (2, c)] for c in range(3)]     # cols
                    + [[(0, 0), (1, 1), (2, 2)],                       # diag
                       [(0, 2), (1, 1), (2, 0)]]                       # anti
                )
                M = [wp.tile([P, nb, NWIN, NWIN], F32, tag=f"M{g}", name=f"M{g}")
                     for g in range(8)]
                SS = [wp.tile([P, nb, NWIN, NWIN], F32, tag=f"SS{g}", name=f"SS{g}")
                      for g in range(8)]
                for g, mem in enumerate(GROUPS):
                    mb = M[g][:, :sn]
                    (r0, c0), (r1, c1), (r2, c2) = mem
                    nc.vector.tensor_add(
                        mb, Y[r0][c0][:, :sn], Y[r1][c1][:, :sn])
                    nc.vector.tensor_add(mb, mb, Y[r2][c2][:, :sn])
                    # Msq = (M/3)^2 in place
                    nc.scalar.activation(mb, mb, ACTF.Square, scale=1.0 / 3)
                # squares of Y in place
                for r in range(3):
                    for c in range(3):
                        yb = Y[r][c][:, :sn]
                        nc.scalar.activation(yb, yb, ACTF.Square)
                for g, mem in enumerate(GROUPS):
                    sb = SS[g][:, :sn]
                    (r0, c0), (r1, c1), (r2, c2) = mem
                    nc.vector.tensor_add(
                        sb, Y[r0][c0][:, :sn], Y[r1][c1][:, :sn])
                    nc.vector.tensor_add(sb, sb, Y[r2][c2][:, :sn])
                    # std^2 = SS/3 - (M/3)^2, clamp, sqrt -> sigma in SS tile
                    # (sqrt via exp(0.5*ln x): keeps every ACT func in the
                    # natural_log_exp_and_others table set -- no table swaps)
                    nc.vector.scalar_tensor_tensor(
                        sb, sb, 1.0 / 3, M[g][:, :sn], ALU.mult, ALU.subtract)
                    nc.vector.tensor_scalar_max(sb, sb, 1e-38)
                    nc.scalar.activation(sb, sb, ACTF.Ln)
                    nc.scalar.activation(sb, sb, ACTF.Exp, scale=0.5)

                U1 = wp.tile([P, nb, NWIN, NWIN], F32, tag="U1", name="U1")
                U2 = wp.tile([P, nb, NWIN, NWIN], F32, tag="U2", name="U2")
                t1 = wp.tile([P, nb, NWIN, NWIN], F32, tag="t1", name="t1")
                t2 = wp.tile([P, nb, NWIN, NWIN], F32, tag="t2", name="t2")
                A = wp.tile([P, nb, NWIN, NWIN], F32, tag="A", name="A")
                sum2 = wp.tile([P, nb, NWIN, NWIN], F32, tag="sum2", name="sum2")
                aq = wp.tile([P, nb, NWIN, NWIN], F32, tag="aq", name="aq")
                s_t = wp.tile([P, nb, NWIN, NWIN], F32, tag="s_t", name="s_t")
                ssq = wp.tile([P, nb, NWIN, NWIN], F32, tag="ssq", name="ssq")
                rinv = wp.tile([P, nb, NWIN, NWIN], F32, tag="rinv", name="rinv")
                psi = wp.tile([P, nb, NWIN, NWIN], F32, tag="psi", name="psi")
                u1, u2 = U1[:, :sn], U2[:, :sn]
                tb1, tb2 = t1[:, :sn], t2[:, :sn]
                ab = A[:, :sn]
                s2b, aqb = sum2[:, :sn], aq[:, :sn]
                stb, ssqb, rb, psib = (s_t[:, :sn], ssq[:, :sn],
                                       rinv[:, :sn], psi[:, :sn])
                sig = [SS[g][:, :sn] for g in range(8)]

                nc.vector.tensor_add(u1, sig[0], sig[1])
                nc.vector.tensor_add(u1, u1, sig[2])
                nc.vector.tensor_add(u2, sig[3], sig[4])
                nc.vector.tensor_add(u2, u2, sig[5])
                # A = U1/3 + U2/3 + sig6 + sig7
                nc.vector.scalar_tensor_tensor(
                    tb1, u1, 1.0 / 3, sig[6], ALU.mult, ALU.add)
                nc.vector.scalar_tensor_tensor(
                    tb2, u2, 1.0 / 3, sig[7], ALU.mult, ALU.add)
                nc.vector.tensor_add(ab, tb1, tb2)
                # sum of squared directional stds
                nc.scalar.activation(u1, u1, ACTF.Square, scale=1.0 / 3)
                nc.scalar.activation(u2, u2, ACTF.Square, scale=1.0 / 3)
                nc.scalar.activation(sig[6], sig[6], ACTF.Square)
                nc.scalar.activation(sig[7], sig[7], ACTF.Square)
                nc.vector.tensor_add(tb1, u1, u2)
                nc.vector.tensor_add(tb2, sig[6], sig[7])
                nc.vector.tensor_add(s2b, tb1, tb2)
                # psi = (sum2 - A^2/4) / (3 * (A/4 + 1e-8)^2)
                nc.scalar.activation(aqb, ab, ACTF.Square, scale=0.5)
                nc.vector.tensor_sub(s2b, s2b, aqb)
                nc.vector.tensor_scalar(
                    stb, ab, 0.25, 1e-8, ALU.mult, ALU.add)
                nc.scalar.activation(ssqb, stb, ACTF.Square)
                nc.vector.reciprocal(rb, ssqb)
                nc.vector.scalar_tensor_tensor(
                    psib, s2b, 1.0 / 3, rb, ALU.mult, ALU.mult)
                # psi_m accumulate per seg
                for i in range(sn):
                    s = s0 + i
                    nc.vector.tensor_scalar(
                        pdum, psib[:, i].rearrange("p i j -> p (i j)"),
                        1.0, None, ALU.mult, ALU.add,
                        accum_out=psi_acc[:, s:s + 1])

            wp_ctx.__exit__(None, None, None)
            ep_ctx = tc.tile_pool(name="ent", bufs=1)
            ep = ep_ctx.__enter__()
            # ---- quantize: pi = round(x*255) via the 2^23 RNE trick ----
            # PI2: per seg the 196 pixel codes stored twice (j and j+196) so
            # circularly shifted reads stay within the seg row.  PI2o: the
            # same, rotated by one pixel, so odd shifts read at even (4B)
            # offsets and keep the DVE 2x mode.
            PI2 = ep.tile([P, segs, 2 * NPIX], BF16)
            PI2o = ep.tile([P, segs, 2 * NPIX], BF16)
            TWO23 = float(2 ** 23)
            qch = (segs // 8) * NPIX
            TMP3 = TMP.rearrange("p (s k) -> p s k", k=NPIX)
            spq = segs // 8
            for q in range(8):
                nc.vector.tensor_scalar(
                    TMP, Xf[:, q * qch:(q + 1) * qch], 255.0, TWO23,
                    ALU.mult, ALU.add)
                nc.vector.tensor_scalar(
                    PI2[:, q * spq:(q + 1) * spq, 0:NPIX], TMP3, TWO23,
                    None, ALU.subtract)
            nc.vector.tensor_copy(PI2[:, :, NPIX:2 * NPIX],
                                  PI2[:, :, 0:NPIX])
            nc.vector.tensor_copy(PI2o[:, :, 0:2 * NPIX - 1],
                                  PI2[:, :, 1:2 * NPIX])
            nc.vector.tensor_copy(PI2o[:, :, 2 * NPIX - 1:2 * NPIX],
                                  PI2[:, :, 1:2])

            # ---- entropy: per-pixel own-bin counts via 195 shifted
            # equality passes (all segs per instruction), then
            # E = log2(N) - mean_p ln(count_p) / ln 2 ----
            ACC = ep.tile([P, segs, NPIX], BF16)
            EQT = ep.tile([P, segs, NPIX], BF16)
            base = PI2[:, :, 0:NPIX]
            ACCf = ACC.rearrange("p s k -> p (s k)")
            EQTf = EQT.rearrange("p s k -> p (s k)")
            nc.vector.tensor_tensor(ACC, base, PI2o[:, :, 0:NPIX],
                                    ALU.is_equal)
            for s in range(2, NPIX):
                if s % 2 == 0:
                    shifted = PI2[:, :, s:s + NPIX]
                else:
                    shifted = PI2o[:, :, s - 1:s - 1 + NPIX]
                nc.vector.tensor_tensor(EQT, base, shifted, ALU.is_equal)
                nc.vector.tensor_tensor(ACC, ACC, EQT, ALU.add)
            # ln(count) with the +1 self-match folded into the ACT bias
            LNP = ep.tile([P, segs, NPIX], F32)
            LNPf = LNP.rearrange("p s k -> p (s k)")
            nc.scalar.activation(LNPf, ACCf, ACTF.Ln, bias=1.0)
            for s in range(segs):
                nc.vector.tensor_scalar(
                    dummy, LNP[:, s], 1.0, None, ALU.mult,
                    ALU.add, accum_out=e_acc[:, s:s + 1])

            ep_ctx.__exit__(None, None, None)
            # ---- richness = psi_m * entropy, mean over channels ----
            import math
            nc.vector.tensor_scalar(
                e_acc, e_acc, -1.0 / (NPIX * LN2), float(math.log2(NPIX)),
                ALU.mult, ALU.add)
            nc.vector.scalar_tensor_tensor(
                rich, psi_acc, 1.0 / (NWIN * NWIN), e_acc,
                ALU.mult, ALU.mult)
            nc.vector.tensor_add(tsum, rich3[:, :, 0], rich3[:, :, 1])
            nc.vector.tensor_add(tsum, tsum, rich3[:, :, 2])
            nc.vector.tensor_scalar(osb, tsum, 1.0 / C, None, ALU.mult)

            # ---- output DMAs ----
            for t in range(T_BLKS):
                b = t // (T_BLKS // B_CORE)
                hp0 = (t % (T_BLKS // B_CORE)) * 4
                nc.sync.dma_start(ov[b, hp0:hp0 + 4], osb[:, t:t + 1])

    nc.compile()
    return nc


@functools.lru_cache(maxsize=4)
def _build_cached(dct_flat: tuple) -> bass.Bass:
    return _build(dct_flat)


def kernel(x, dct_matrix):
    x = np.ascontiguousarray(np.asarray(x, dtype=np.float32))
    D = np.asarray(dct_matrix, dtype=np.float32)
    assert x.shape == (B_FULL, C, H, W), x.shape
    nc = _build_cached(tuple(float(v) for v in D.flatten()))
    in_maps = [
        {"x": np.ascontiguousarray(x[i * B_CORE:(i + 1) * B_CORE])}
        for i in range(N_CORES)
    ]
    res = bass_utils.run_bass_kernel_spmd(
        nc, in_maps, core_ids=list(range(N_CORES)))
    out = np.concatenate([r["out"] for r in res.results], axis=0)
    return out.astype(np.float32)


# revision 30
# speedup vs baseline: 980.7263x; 636.6628x over previous
"""Trainium2 Bass kernel for nn_DirectionVarEntropy.

Computes, per 14x14 patch and channel:
  - pixel-value entropy (256-bin histogram of round(x*255))
  - direction variance psi of 3x3-DCT sliding-window directional stds
  - richness = mean_c(psi_m * entropy)  ->  output (B, Hp, Wp)

Sharding: pure data parallel over batch, 2 images per core on 8 cores.

Per-core layout: 2048 spatial patches x 3 channels = 6144 patch-channels,
mapped to [128 partitions x 48 free segments]; seg s = t*3 + c where
t = spatial_patch // 128, partition p = spatial_patch % 128.

Entropy (the histogram_binning part): instead of materializing 256-bin
histograms (which needs either scatter-add hardware this chip lacks, or
256 compare+reduce passes dominated by per-instruction overhead), compute
per-pixel own-bin counts c_p = #\{q: pi_q == pi_p\} with 195 circular-shift
tensor_tensor(is_equal) + add passes in bf16 (DVE 2x mode), each one
instruction covering all 48 segments.  Then
  E = log2(196) - mean_p ln(c_p)/ln 2
which equals the dense-histogram entropy up to the reference's 1e-10
epsilon terms (~1e-6 relative).  Shifted reads stay 4B-aligned via two
doubled pixel buffers (one rotated by a pixel) so the DVE keeps its fast
mode for odd shifts.

DCT part: explicit 9 coefficient planes via separable 3-tap convolutions
(tensor_scalar + scalar_tensor_tensor on shifted access patterns), group
sums / stds / psi in fp32 on DVE; ACT does squares and sqrt via
exp(0.5*ln x) so every activation stays in one LUT function-set (no
1.3us table reloads).  SBUF is phase-scoped: conv/psi blocks run first
(X + work pool), then the entropy phase reuses that space.
"""

import functools

import numpy as np

import concourse.bacc as bacc
import concourse.bass as bass
import concourse.mybir as mybir
from concourse import bass_utils
from concourse.tile import TileContext

P = 128
PH = 14
NWIN = 12          # sliding 3x3 positions per axis
NPIX = PH * PH     # 196
BINS = 256
LN2 = 0.6931471805599453
F32 = mybir.dt.float32
BF16 = mybir.dt.bfloat16
ALU = mybir.AluOpType
ACTF = mybir.ActivationFunctionType

# problem shape (hardcoded per contract)
B_FULL, C, H, W = 16, 3, 448, 448
N_CORES = 8
B_CORE = B_FULL // N_CORES      # 2
HP = H // PH                    # 32
T_BLKS = B_CORE * HP * HP // P  # 16 t-blocks of 128 spatial patches
SEGS = T_BLKS * C               # 48


def _build(dct_flat: tuple, segs: int = SEGS, bins: int = BINS,
           nb: int = 3, act_bins: int = 0) -> bass.Bass:
    """Build the SPMD single-core program. dct_flat: 9 floats, row major."""
    D = np.asarray(dct_flat, np.float64).reshape(3, 3)
    nc = bacc.Bacc("TRN2", debug=False, enable_asserts=False)

    x_d = nc.dram_tensor("x", (B_CORE, C, H, W), F32, kind="ExternalInput")
    out_d = nc.dram_tensor("out", (B_CORE, HP, HP), F32, kind="ExternalOutput")
    # (b, c, hp, i, wp, j) view of DRAM input, reordered to (b c hp wp i j)
    xv = x_d.ap().rearrange("b c (hp i) (wp j) -> b c hp wp i j", i=PH, j=PH)
    ov = out_d.ap()

    n_blocks = (segs + nb - 1) // nb

    with TileContext(nc) as tc:
        with tc.tile_pool(name="persist", bufs=1) as pp:
            X = pp.tile([P, segs, PH, PH], F32)
            Xf = X.rearrange("p s i j -> p (s i j)")
            TMP = pp.tile([P, (segs // 8) * NPIX], F32)
            dummy = pp.tile([P, NPIX], BF16)
            pdum = pp.tile([P, NWIN * NWIN], F32)
            psi_acc = pp.tile([P, segs], F32)
            e_acc = pp.tile([P, segs], F32)
            rich = pp.tile([P, segs], F32)
            rich3 = rich.rearrange("p (t c) -> p t c", c=C)
            tsum = pp.tile([P, segs // C], F32)
            osb = pp.tile([P, segs // C], F32)

            # ---- input DMAs: per (t, c, p1) a [32, 14, 14] strided load ----
            for t in range(T_BLKS):
                b = t // (T_BLKS // B_CORE)
                hp0 = (t % (T_BLKS // B_CORE)) * 4
                for c in range(C):
                    s = t * C + c
                    for p1 in range(4):
                        nc.sync.dma_start(
                            X[p1 * 32:(p1 + 1) * 32, s],
                            xv[b, c, hp0 + p1],
                        )
            # Per-DMA same-engine absorber copies: each waits on exactly one
            # DMA queue semaphore; all downstream DVE reads of X then order
            # behind these in program order (no multi-sem waits, which
            # overflow the ISA sync-wait slots).
            for t in range(T_BLKS):
                for c in range(C):
                    s = t * C + c
                    for p1 in range(4):
                        sl = X[p1 * 32:(p1 + 1) * 32, s]
                        nc.vector.tensor_copy(sl, sl)

            d = [[float(D[r, c]) for c in range(3)] for r in range(3)]

            wp_ctx = tc.tile_pool(name="work", bufs=2)
            wp = wp_ctx.__enter__()
            for blk in range(n_blocks):
                s0 = blk * nb
                sn = min(nb, segs - s0)
                # conv tiles for this block
                V = [wp.tile([P, nb, NWIN, PH], F32, tag=f"V{r}", name=f"V{r}")
                     for r in range(3)]
                Y = [[wp.tile([P, nb, NWIN, NWIN], F32, tag=f"Y{r}{c}", name=f"Y{r}{c}")
                      for c in range(3)] for r in range(3)]
                xb = X[:, s0:s0 + sn]

                # vertical convs V_r(i,j) = sum_k D[r,k] x(i+k, j)
                for r in range(3):
                    vb = V[r][:, :sn]
                    nc.vector.tensor_scalar(
                        vb, xb[:, :, 0:NWIN, :], d[r][0], None, ALU.mult)
                    for k in (1, 2):
                        nc.vector.scalar_tensor_tensor(
                            vb, xb[:, :, k:k + NWIN, :], d[r][k], vb,
                            ALU.mult, ALU.add)
                # horizontal convs Y_rc(i,j) = sum_l D[c,l] V_r(i, j+l)
                for r in range(3):
                    vb = V[r][:, :sn]
                    for c in range(3):
                        yb = Y[r][c][:, :sn]
                        nc.vector.tensor_scalar(
                            yb, vb[:, :, :, 0:NWIN], d[c][0], None, ALU.mult)
                        for l in (1, 2):
                            nc.vector.scalar_tensor_tensor(
                                yb, vb[:, :, :, l:l + NWIN], d[c][l], yb,
                                ALU.mult, ALU.add)

                # group sums of Y (pre-square): rows, cols, diag, anti-diag
                GROUPS = (
                    [[(r, 0), (r, 1), (r, 2)] for r in range(3)]       # rows
                    + [[(0, c), (1, c), (2, c)] for c in range(3)]     # cols
                    + [[(0, 0), (1, 1), (2, 2)],                       # diag
                       [(0, 2), (1, 1), (2, 0)]]                       # anti
                )
                M = [wp.tile([P, nb, NWIN, NWIN], F32, tag=f"M{g}", name=f"M{g}")
                     for g in range(8)]
                SS = [wp.tile([P, nb, NWIN, NWIN], F32, tag=f"SS{g}", name=f"SS{g}")
                      for g in range(8)]
                for g, mem in enumerate(GROUPS):
                    mb = M[g][:, :sn]
                    (r0, c0), (r1, c1), (r2, c2) = mem
                    nc.vector.tensor_add(
                        mb, Y[r0][c0][:, :sn], Y[r1][c1][:, :sn])
                    nc.vector.tensor_add(mb, mb, Y[r2][c2][:, :sn])
                    # Msq = (M/3)^2 in place
                    nc.scalar.activation(mb, mb, ACTF.Square, scale=1.0 / 3)
                # squares of Y in place
                for r in range(3):
                    for c in range(3):
                        yb = Y[r][c][:, :sn]
                        nc.scalar.activation(yb, yb, ACTF.Square)
                for g, mem in enumerate(GROUPS):
                    sb = SS[g][:, :sn]
                    (r0, c0), (r1, c1), (r2, c2) = mem
                    nc.vector.tensor_add(
                        sb, Y[r0][c0][:, :sn], Y[r1][c1][:, :sn])
                    nc.vector.tensor_add(sb, sb, Y[r2][c2][:, :sn])
                    # std^2 = SS/3 - (M/3)^2, clamp, sqrt -> sigma in SS tile
                    # (sqrt via exp(0.5*ln x): keeps every ACT func in the
                    # natural_log_exp_and_others table set -- no table swaps)
                    nc.vector.scalar_tensor_tensor(
                        sb, sb, 1.0 / 3, M[g][:, :sn], ALU.mult, ALU.subtract)
                    nc.vector.tensor_scalar_max(sb, sb, 1e-38)
                    nc.scalar.activation(sb, sb, ACTF.Ln)
                    nc.scalar.activation(sb, sb, ACTF.Exp, scale=0.5)

                U1 = wp.tile([P, nb, NWIN, NWIN], F32, tag="U1", name="U1")
                U2 = wp.tile([P, nb, NWIN, NWIN], F32, tag="U2", name="U2")
                t1 = wp.tile([P, nb, NWIN, NWIN], F32, tag="t1", name="t1")
                t2 = wp.tile([P, nb, NWIN, NWIN], F32, tag="t2", name="t2")
                A = wp.tile([P, nb, NWIN, NWIN], F32, tag="A", name="A")
                sum2 = wp.tile([P, nb, NWIN, NWIN], F32, tag="sum2", name="sum2")
                aq = wp.tile([P, nb, NWIN, NWIN], F32, tag="aq", name="aq")
                s_t = wp.tile([P, nb, NWIN, NWIN], F32, tag="s_t", name="s_t")
                ssq = wp.tile([P, nb, NWIN, NWIN], F32, tag="ssq", name="ssq")
                rinv = wp.tile([P, nb, NWIN, NWIN], F32, tag="rinv", name="rinv")
                psi = wp.tile([P, nb, NWIN, NWIN], F32, tag="psi", name="psi")
                u1, u2 = U1[:, :sn], U2[:, :sn]
                tb1, tb2 = t1[:, :sn], t2[:, :sn]
                ab = A[:, :sn]
                s2b, aqb = sum2[:, :sn], aq[:, :sn]
                stb, ssqb, rb, psib = (s_t[:, :sn], ssq[:, :sn],
                                       rinv[:, :sn], psi[:, :sn])
                sig = [SS[g][:, :sn] for g in range(8)]

                nc.vector.tensor_add(u1, sig[0], sig[1])
                nc.vector.tensor_add(u1, u1, sig[2])
                nc.vector.tensor_add(u2, sig[3], sig[4])
                nc.vector.tensor_add(u2, u2, sig[5])
                # A = U1/3 + U2/3 + sig6 + sig7
                nc.vector.scalar_tensor_tensor(
                    tb1, u1, 1.0 / 3, sig[6], ALU.mult, ALU.add)
                nc.vector.scalar_tensor_tensor(
                    tb2, u2, 1.0 / 3, sig[7], ALU.mult, ALU.add)
                nc.vector.tensor_add(ab, tb1, tb2)
                # sum of squared directional stds
                nc.scalar.activation(u1, u1, ACTF.Square, scale=1.0 / 3)
                nc.scalar.activation(u2, u2, ACTF.Square, scale=1.0 / 3)
                nc.scalar.activation(sig[6], sig[6], ACTF.Square)
                nc.scalar.activation(sig[7], sig[7], ACTF.Square)
                nc.vector.tensor_add(tb1, u1, u2)
                nc.vector.tensor_add(tb2, sig[6], sig[7])
                nc.vector.tensor_add(s2b, tb1, tb2)
                # psi = (sum2 - A^2/4) / (3 * (A/4 + 1e-8)^2)
                nc.scalar.activation(aqb, ab, ACTF.Square, scale=0.5)
                nc.vector.tensor_sub(s2b, s2b, aqb)
                nc.vector.tensor_scalar(
                    stb, ab, 0.25, 1e-8, ALU.mult, ALU.add)
                nc.scalar.activation(ssqb, stb, ACTF.Square)
                nc.vector.reciprocal(rb, ssqb)
                nc.vector.scalar_tensor_tensor(
                    psib, s2b, 1.0 / 3, rb, ALU.mult, ALU.mult)
                # psi_m accumulate per seg
                for i in range(sn):
                    s = s0 + i
                    nc.vector.tensor_scalar(
                        pdum, psib[:, i].rearrange("p i j -> p (i j)"),
                        1.0, None, ALU.mult, ALU.add,
                        accum_out=psi_acc[:, s:s + 1])

            wp_ctx.__exit__(None, None, None)
            ep_ctx = tc.tile_pool(name="ent", bufs=1)
            ep = ep_ctx.__enter__()
            # ---- quantize: pi = round(x*255) via the 2^23 RNE trick ----
            # PI2: per seg the 196 pixel codes stored twice (j and j+196) so
            # circularly shifted reads stay within the seg row.  PI2o: the
            # same, rotated by one pixel, so odd shifts read at even (4B)
            # offsets and keep the DVE 2x mode.
            PI2 = ep.tile([P, segs, 2 * NPIX], BF16)
            PI2o = ep.tile([P, segs, 2 * NPIX], BF16)
            TWO23 = float(2 ** 23)
            qch = (segs // 8) * NPIX
            TMP3 = TMP.rearrange("p (s k) -> p s k", k=NPIX)
            spq = segs // 8
            for q in range(8):
                nc.vector.tensor_scalar(
                    TMP, Xf[:, q * qch:(q + 1) * qch], 255.0, TWO23,
                    ALU.mult, ALU.add)
                nc.vector.tensor_scalar(
                    PI2[:, q * spq:(q + 1) * spq, 0:NPIX], TMP3, TWO23,
                    None, ALU.subtract)
            nc.vector.tensor_copy(PI2[:, :, NPIX:2 * NPIX],
                                  PI2[:, :, 0:NPIX])
            nc.vector.tensor_copy(PI2o[:, :, 0:2 * NPIX - 1],
                                  PI2[:, :, 1:2 * NPIX])
            nc.vector.tensor_copy(PI2o[:, :, 2 * NPIX - 1:2 * NPIX],
                                  PI2[:, :, 1:2])

            # ---- entropy: per-pixel own-bin counts via 195 shifted
            # equality passes (all segs per instruction), then
            # E = log2(N) - mean_p ln(count_p) / ln 2 ----
            ACC = ep.tile([P, segs, NPIX], BF16)
            EQT = ep.tile([P, segs, NPIX], BF16)
            base = PI2[:, :, 0:NPIX]
            ACCf = ACC.rearrange("p s k -> p (s k)")
            EQTf = EQT.rearrange("p s k -> p (s k)")
            nc.vector.tensor_tensor(ACC, base, PI2o[:, :, 0:NPIX],
                                    ALU.is_equal)
            for s in range(2, NPIX):
                if s % 2 == 0:
                    shifted = PI2[:, :, s:s + NPIX]
                else:
                    shifted = PI2o[:, :, s - 1:s - 1 + NPIX]
                nc.vector.tensor_tensor(EQT, base, shifted, ALU.is_equal)
                nc.vector.tensor_tensor(ACC, ACC, EQT, ALU.add)
            # ln(count) with the +1 self-match folded into the ACT bias
            LNP = ep.tile([P, segs, NPIX], F32)
            LNPf = LNP.rearrange("p s k -> p (s k)")
            nc.scalar.activation(LNPf, ACCf, ACTF.Ln, bias=1.0)
            for s in range(segs):
                nc.vector.tensor_scalar(
                    dummy, LNP[:, s], 1.0, None, ALU.mult,
                    ALU.add, accum_out=e_acc[:, s:s + 1])

            ep_ctx.__exit__(None, None, None)
            # ---- richness = psi_m * entropy, mean over channels ----
            import math
            nc.vector.tensor_scalar(
                e_acc, e_acc, -1.0 / (NPIX * LN2), float(math.log2(NPIX)),
                ALU.mult, ALU.add)
            nc.vector.scalar_tensor_tensor(
                rich, psi_acc, 1.0 / (NWIN * NWIN), e_acc,
                ALU.mult, ALU.mult)
            nc.vector.tensor_add(tsum, rich3[:, :, 0], rich3[:, :, 1])
            nc.vector.tensor_add(tsum, tsum, rich3[:, :, 2])
            nc.vector.tensor_scalar(osb, tsum, 1.0 / C, None, ALU.mult)

            # ---- output DMAs ----
            for t in range(T_BLKS):
                b = t // (T_BLKS // B_CORE)
                hp0 = (t % (T_BLKS // B_CORE)) * 4
                nc.sync.dma_start(ov[b, hp0:hp0 + 4], osb[:, t:t + 1])

    nc.compile()
    return nc


@functools.lru_cache(maxsize=4)
def _build_cached(dct_flat: tuple) -> bass.Bass:
    return _build(dct_flat)


def kernel(x, dct_matrix):
    x = np.ascontiguousarray(np.asarray(x, dtype=np.float32))
    D = np.asarray(dct_matrix, dtype=np.float32)
    assert x.shape == (B_FULL, C, H, W), x.shape
    nc = _build_cached(tuple(float(v) for v in D.flatten()))
    in_maps = [
        {"x": np.ascontiguousarray(x[i * B_CORE:(i + 1) * B_CORE])}
        for i in range(N_CORES)
    ]
    res = bass_utils.run_bass_kernel_spmd(
        nc, in_maps, core_ids=list(range(N_CORES)))
    out = np.concatenate([r["out"] for r in res.results], axis=0)
    return out.astype(np.float32)
